# revision 1
# baseline (speedup 1.0000x reference)
"""Trainium2 Bass kernel for an attention block (B=8, T=2048, D=K=V=1024).

Reference math (per batch element, sharded one per NeuronCore):
    Q = x @ Wq.T + bq ; K = x @ Wk.T + bk ; V = x @ Wv.T + bv
    logits[t,s] = Q[t] . K[s],  masked -inf for s > t (strict upper tri)
    probs = softmax(logits, axis=t) / sqrt(1024)     # softmax over QUERY axis
    out = x + probs @ V

Implementation notes:
  - Everything is computed in a transposed layout: QT/KT are [k, t] (k on
    partitions) so logitsT = [s, t] comes straight out of the PE, and the
    softmax reduction (over t) is a free-axis reduction.
  - softmax over t for fixed s:  P[s,t] = exp(l[s,t]);  Z[s] = sum_t P[s,t];
    read[t,v] = sum_s P[s,t] * V[s,v] / (32 * Z[s]).  No max subtraction is
    needed: logits are ~N(0,13^2), max |l| < ~75 so exp stays inside fp32.
  - Matmuls run as float32r (full PE rate at free dim 512). P and V are
    stored bf16 for the PV matmul (also full rate).
  - Causal structure skips fully-masked tiles; diagonal 128x512 tiles get an
    additive -1e30 staircase mask.
"""

import time

import numpy as np

import concourse.bass as bass
import concourse.bacc as bacc
import concourse.mybir as mybir
import concourse.tile as tile
from concourse.bass_utils import run_bass_kernel_spmd
from concourse.masks import make_identity

F32 = mybir.dt.float32
F32R = mybir.dt.float32r
BF16 = mybir.dt.bfloat16
AF = mybir.ActivationFunctionType

P = 128          # partitions
T = 2048         # sequence length
D = 1024         # model dim
TB = 512         # t-block width
NTB = T // TB    # 4 t-blocks
DK = D // P      # 8 contraction subtiles
KO = D // P      # 8 k output tiles
SV = T // P      # 16 s tiles
NEG = -1.0e30


def _transpose_weight(nc, tc, pools, w_ap, dst):
    """Transpose a [1024, 1024] DRAM weight into dst SBUF tile [128, 8, 1024]
    laid out as dst[d_inner, d_outer, k]."""
    wnat_pool, psum_t, identity = pools
    for kt in range(8):
        wnat = wnat_pool.tile([P, D], F32R, name="wnat", tag="wnat")
        eng_a = nc.gpsimd if kt % 2 == 0 else nc.sync
        eng_b = nc.sync if kt % 2 == 0 else nc.gpsimd
        eng_a.dma_start(out=wnat[:P // 2, :],
                        in_=w_ap[kt * P:kt * P + P // 2, :].bitcast(F32R))
        eng_b.dma_start(out=wnat[P // 2:, :],
                        in_=w_ap[kt * P + P // 2:(kt + 1) * P, :].bitcast(F32R))
        for dk in range(DK):
            pt = psum_t.tile([P, P], F32R, name="pt", tag="pt")
            nc.tensor.transpose(
                pt,
                wnat[:, dk * P:(dk + 1) * P],
                identity,
            )
            nc.vector.tensor_copy(out=dst[:, dk, kt * P:(kt + 1) * P], in_=pt)


def _build_nc():
    nc = bacc.Bacc("TRN2", target_bir_lowering=False, debug=False, num_devices=8)

    x = nc.dram_tensor("x", [T, D], F32, kind="ExternalInput").ap()
    Wq = nc.dram_tensor("Wq", [D, D], F32, kind="ExternalInput").ap()
    bq = nc.dram_tensor("bq", [D], F32, kind="ExternalInput").ap()
    Wk = nc.dram_tensor("Wk", [D, D], F32, kind="ExternalInput").ap()
    bk = nc.dram_tensor("bk", [D], F32, kind="ExternalInput").ap()
    Wv = nc.dram_tensor("Wv", [D, D], F32, kind="ExternalInput").ap()
    bv = nc.dram_tensor("bv", [D], F32, kind="ExternalInput").ap()
    out = nc.dram_tensor("out", [T, D], F32, kind="ExternalOutput").ap()

    with tile.TileContext(nc) as tc:
        _kernel_body(nc, tc, x, Wq, bq, Wk, bk, Wv, bv, out)

    nc.compile()
    return nc


def _kernel_body(nc, tc, x, Wq, bq, Wk, bk, Wv, bv, out):
    from contextlib import ExitStack

    ctx = ExitStack()
    with ctx:
        consts = ctx.enter_context(tc.tile_pool(name="consts", bufs=1))
        wpool = ctx.enter_context(tc.tile_pool(name="wpool", bufs=2))
        ktpool = ctx.enter_context(tc.tile_pool(name="ktpool", bufs=1))
        dram = ctx.enter_context(tc.tile_pool(name="dram", bufs=1, space="DRAM"))
        psum_t = ctx.enter_context(tc.tile_pool(name="psum_t", bufs=3, space="PSUM"))
        psum_mm = ctx.enter_context(tc.tile_pool(name="psum_mm", bufs=5, space="PSUM"))

        # ---- constants ----
        # identity first: it gates every PE transpose at kernel start
        id_f32 = consts.tile([P, P], F32, name="id_f32")
        make_identity(nc, id_f32)
        identity = consts.tile([P, P], F32R, name="identity")
        nc.vector.tensor_copy(out=identity, in_=id_f32)

        # staircase masks for the 4 diagonal positions of a [128 s, 512 t]
        # tile with offset o = s0 - t0 in {0,128,256,384}: valid iff f >= p + o
        # (masks / biases / Z built after the first x-transposes are queued,
        # so they don't block the gpsimd DMA queue at kernel start)
        rtile = consts.tile([P, SV], F32, name="rtile")

        # persistent KT [k_inner, k_outer, s]
        KT = ktpool.tile([P, KO, T], F32R, name="KT", tag="big")

        # DRAM scratch
        xT_dram = dram.tile([NTB, P, DK, TB], F32R, name="xT_dram")
        P_dram = dram.tile([P, SV, SV, P], BF16, name="P_dram")

        with (
            tc.tile_pool(name="wnat", bufs=3) as wnat_pool,
            tc.tile_pool(name="xnat", bufs=3) as xnat_pool,
            tc.tile_pool(name="xtp", bufs=2) as xtp,
            tc.tile_pool(name="qtp", bufs=1) as qtp,
            tc.tile_pool(name="pstage", bufs=2) as pstage,
        ):
            tpools = (wnat_pool, psum_t, identity)

            def make_xT_blk(j):
                xT_blk = xtp.tile([P, DK, TB], F32R, name="xT_blk", tag="xT")
                for ts_ in range(TB // P):
                    t0 = j * TB + ts_ * P
                    xnat = xnat_pool.tile([P, D], F32R, name="xnat", tag="xnat")
                    # split by partition: cuts the per-transfer descriptor
                    # count (DMA engines are descriptor-rate-bound) and runs
                    # the pieces on parallel HW queues. gpsimd ring only
                    # (sync is busy with P/xT writes that wait on compute).
                    # The very first tiles use quarter-splits so the first PE
                    # transpose can start as early as possible.
                    nsplit = 4 if (j == 0 and ts_ < 2) else 2
                    step = P // nsplit
                    for q in range(nsplit):
                        nc.gpsimd.dma_start(
                            out=xnat[q * step:(q + 1) * step, :],
                            in_=x[t0 + q * step:t0 + (q + 1) * step, :].bitcast(F32R))
                    for dk in range(DK):
                        pt = psum_t.tile([P, P], F32R, name="pt", tag="pt")
                        nc.tensor.transpose(
                            pt,
                            xnat[:, dk * P:(dk + 1) * P],
                            identity,
                        )
                        nc.vector.tensor_copy(
                            out=xT_blk[:, dk, ts_ * P:(ts_ + 1) * P], in_=pt
                        )
                nc.sync.dma_start(out=xT_dram[j], in_=xT_blk)
                return xT_blk

            # j=0's x transposes run first: x tiles arrive long before the
            # full weight matrices, so this keeps the PE busy from ~2us.
            xT_first = make_xT_blk(0)

            # one sliding mask [128, 896]: valid (0.0) iff g >= p + 384,
            # else -1e30. mask for diagonal offset oi*128 is the slice
            # [384-128*oi : 896-128*oi].
            mask_base = consts.tile([P, TB + 3 * P], BF16, name="mask_base")
            nc.gpsimd.memset(mask_base, 0.0)
            nc.gpsimd.affine_select(
                out=mask_base, in_=mask_base,
                compare_op=mybir.AluOpType.is_ge,
                fill=NEG,
                base=-(3 * P),
                pattern=[[1, TB + 3 * P]],
                channel_multiplier=-1,
            )
            masks = [mask_base[:, 3 * P - oi * P: 3 * P - oi * P + TB]
                     for oi in range(4)]

            # biases: bq/bk striped [128, 8] (per-partition, k-major);
            # bv broadcast to all partitions [128, 1024]
            bq_sb = consts.tile([P, KO], F32, name="bq_sb")
            nc.sync.dma_start(out=bq_sb, in_=bq.rearrange("(o p) -> p o", p=P))
            bk_sb = consts.tile([P, KO], F32, name="bk_sb")
            nc.sync.dma_start(out=bk_sb, in_=bk.rearrange("(o p) -> p o", p=P))
            bv_sb = consts.tile([P, D], BF16, name="bv_sb")
            bv_bcast = bass.AP(tensor=bv.tensor, offset=bv.offset,
                               ap=[[0, P], [1, D]])
            nc.gpsimd.dma_start(out=bv_sb, in_=bv_bcast)

            Zacc = consts.tile([P, SV, NTB], F32, name="Zacc")
            nc.vector.memset(Zacc, 0.0)

            # ---- phase 1: weight transposes for Q, K ----
            WqT = wpool.tile([P, DK, D], F32R, name="WqT", tag="W")
            _transpose_weight(nc, tc, tpools, Wq, WqT)
            WkT = wpool.tile([P, DK, D], F32R, name="WkT", tag="W")
            _transpose_weight(nc, tc, tpools, Wk, WkT)

            # ---- phase 2: fused x-transpose + QT/KT + logits + exp sweep ----
            xT_next = xT_first
            for j in range(NTB):
                xT_blk = xT_next

                # QT block [k_inner, k_outer, t(512)]
                qt_blk = qtp.tile([P, KO, TB], F32R, name="qt_blk", tag="qt")
                for ko in range(KO):
                    ps = psum_mm.tile([P, TB], F32, name="ps_q", tag="mm")
                    for dk in range(DK):
                        nc.tensor.matmul(
                            ps,
                            lhsT=WqT[:, dk, ko * P:(ko + 1) * P],
                            rhs=xT_blk[:, dk, :],
                            start=(dk == 0),
                            stop=(dk == DK - 1),
                        )
                    nc.scalar.activation(
                        qt_blk[:, ko, :], ps, AF.Identity, bias=bq_sb[:, ko:ko + 1]
                    )

                # KT block
                for ko in range(KO):
                    ps = psum_mm.tile([P, TB], F32, name="ps_k", tag="mm")
                    for dk in range(DK):
                        nc.tensor.matmul(
                            ps,
                            lhsT=WkT[:, dk, ko * P:(ko + 1) * P],
                            rhs=xT_blk[:, dk, :],
                            start=(dk == 0),
                            stop=(dk == DK - 1),
                        )
                    nc.scalar.activation(
                        KT[:, ko, j * TB:(j + 1) * TB], ps, AF.Identity,
                        bias=bk_sb[:, ko:ko + 1],
                    )

                # next block's x transposes are emitted mid-block so the PE
                # reaches them long after their xnat DMAs were issued (no
                # boundary stall), hidden between logits tiles
                logits_order = list(range(4 * (j + 1)))
                split = max(0, len(logits_order) - 4)
                for sv in logits_order[:split]:
                    ps = psum_mm.tile([P, TB], F32, name="ps_l", tag="mm")
                    for ko in range(KO):
                        nc.tensor.matmul(
                            ps,
                            lhsT=KT[:, ko, sv * P:(sv + 1) * P],
                            rhs=qt_blk[:, ko, :],
                            start=(ko == 0),
                            stop=(ko == KO - 1),
                        )
                    oi = sv - 4 * j
                    if oi >= 0:
                        nc.vector.tensor_add(out=ps, in0=ps, in1=masks[oi])
                    pst = pstage.tile([P, TB], BF16, name="pst", tag="pst")
                    nc.scalar.activation(
                        pst, ps, AF.Exp, accum_out=Zacc[:, sv, j:j + 1]
                    )
                    nc.sync.dma_start(
                        out=P_dram[:, 4 * j:4 * j + 4, sv, :],
                        in_=pst.rearrange("p (i t) -> p i t", i=4),
                    )
                if j + 1 < NTB:
                    xT_next = make_xT_blk(j + 1)
                for sv in logits_order[split:]:
                    ps = psum_mm.tile([P, TB], F32, name="ps_l", tag="mm")
                    for ko in range(KO):
                        nc.tensor.matmul(
                            ps,
                            lhsT=KT[:, ko, sv * P:(sv + 1) * P],
                            rhs=qt_blk[:, ko, :],
                            start=(ko == 0),
                            stop=(ko == KO - 1),
                        )
                    oi = sv - 4 * j
                    if oi >= 0:
                        nc.vector.tensor_add(out=ps, in0=ps, in1=masks[oi])
                    pst = pstage.tile([P, TB], BF16, name="pst", tag="pst")
                    nc.scalar.activation(
                        pst, ps, AF.Exp, accum_out=Zacc[:, sv, j:j + 1]
                    )
                    nc.sync.dma_start(
                        out=P_dram[:, 4 * j:4 * j + 4, sv, :],
                        in_=pst.rearrange("p (i t) -> p i t", i=4),
                    )

            # ---- Z -> R = 1/(32 Z) ----
            zsum = consts.tile([P, SV], F32, name="zsum")
            nc.vector.reduce_sum(out=zsum, in_=Zacc, axis=mybir.AxisListType.X)
            nc.vector.reciprocal(rtile, zsum)
            nc.vector.tensor_scalar_mul(rtile, rtile, 1.0 / 32.0)

            # ---- phase 3: V' = (x @ Wv.T + bv) / (32 Z), written straight
            # into Vp (which reuses KT's SBUF slot, free after phase 2) ----
            Vp = ktpool.tile([P, SV, D], BF16, name="Vp", tag="big")
            WvT = wpool.tile([P, DK, D], F32R, name="WvT", tag="W")
            _transpose_weight(nc, tc, tpools, Wv, WvT)
            for j in range(NTB):
                xT_blk2 = xtp.tile([P, DK, TB], F32R, name="xT_blk2", tag="xT")
                nc.sync.dma_start(out=xT_blk2, in_=xT_dram[j])
                for si in range(TB // P):
                    sv = j * 4 + si
                    for h in range(D // TB):
                        ps = psum_mm.tile([P, TB], F32, name="ps_v", tag="mm")
                        for dk in range(DK):
                            nc.tensor.matmul(
                                ps,
                                lhsT=xT_blk2[:, dk, si * P:(si + 1) * P],
                                rhs=WvT[:, dk, h * TB:(h + 1) * TB],
                                start=(dk == 0),
                                stop=(dk == DK - 1),
                            )
                        nc.vector.tensor_add(
                            out=Vp[:, sv, h * TB:(h + 1) * TB],
                            in0=ps,
                            in1=bv_sb[:, h * TB:(h + 1) * TB],
                        )
                        nc.vector.tensor_scalar_mul(
                            Vp[:, sv, h * TB:(h + 1) * TB],
                            Vp[:, sv, h * TB:(h + 1) * TB],
                            rtile[:, sv:sv + 1],
                        )

        # ---- phase 4: read = P^T . V', out = x + read ----
        with (
            tc.tile_pool(name="pcol", bufs=3) as pcol_pool,
            tc.tile_pool(name="ost", bufs=2) as ost_pool,
            tc.tile_pool(name="xres", bufs=2) as xres_pool,
        ):
            for i in range(SV):
                pcol = pcol_pool.tile([P, SV, P], BF16, name="pcol", tag="pcol")
                nc.gpsimd.dma_start(
                    out=pcol[:, 0:i + 1, :], in_=P_dram[:, i, 0:i + 1, :]
                )
                xres = xres_pool.tile([P, D], F32, name="xres", tag="xres")
                nc.gpsimd.dma_start(out=xres, in_=x[i * P:(i + 1) * P, :])
                ost = ost_pool.tile([P, D], F32, name="ost", tag="ost")
                for h in range(D // TB):
                    ps = psum_mm.tile([P, TB], F32, name="ps_o", tag="mm")
                    for svv in range(i + 1):
                        nc.tensor.matmul(
                            ps,
                            lhsT=pcol[:, svv, :],
                            rhs=Vp[:, svv, h * TB:(h + 1) * TB],
                            start=(svv == 0),
                            stop=(svv == i),
                        )
                    nc.vector.tensor_add(
                        out=ost[:, h * TB:(h + 1) * TB],
                        in0=ps,
                        in1=xres[:, h * TB:(h + 1) * TB],
                    )
                nc.sync.dma_start(out=out[i * P:(i + 1) * P, :], in_=ost)


_NC_CACHE = None


def _get_nc():
    global _NC_CACHE
    if _NC_CACHE is None:
        _NC_CACHE = _build_nc()
    return _NC_CACHE


def kernel(minibatch, Wq, bq, Wk, bk, Wv, bv):
    minibatch = np.asarray(minibatch, dtype=np.float32)
    Wq = np.asarray(Wq, dtype=np.float32)
    bq = np.asarray(bq, dtype=np.float32)
    Wk = np.asarray(Wk, dtype=np.float32)
    bk = np.asarray(bk, dtype=np.float32)
    Wv = np.asarray(Wv, dtype=np.float32)
    bv = np.asarray(bv, dtype=np.float32)

    nc = _get_nc()
    B = minibatch.shape[0]
    in_maps = [
        {
            "x": np.ascontiguousarray(minibatch[i]),
            "Wq": Wq, "bq": bq, "Wk": Wk, "bk": bk, "Wv": Wv, "bv": bv,
        }
        for i in range(B)
    ]
    last_err = None
    for _attempt in range(3):
        try:
            res = run_bass_kernel_spmd(nc, in_maps, core_ids=list(range(B)))
            break
        except Exception as e:  # transient device errors (e.g. NRT_EXEC_UNIT_UNRECOVERABLE)
            last_err = e
            time.sleep(2.0)
    else:
        raise last_err
    return np.stack([res.results[i]["out"] for i in range(B)], axis=0)

